# revision 29
# baseline (speedup 1.0000x reference)
"""Trainium2 Bass kernel for nn_MultiHeadClassifier (moe_routing).

Strategy: data-parallel over N=32768 points across 8 NeuronCores (4096
points/core), then EXPERT ROUTING within each core: the host stably sorts
each core's points by category and pads every category to CAP=384 slots.
The device then computes, per point, only its own category's 256-channel
block of x1 = feat @ W1 (16x less PE work than the dense 4096-channel
version) plus that category's 6-way head.

BatchNorm batch statistics over the FULL 4096 channels are computed
analytically from the 256x256 feature Gram matrix:
  sumx1[j]  = s^T W1[:, j]          (s = per-feature column sums)
  sumsq[j]  = W1[:, j]^T C W1[:, j] (C = F^T F)
via D_T = W1^T C (with an extra ones-column of the Gram giving s^T W1 for
free), P_T = D_T * W1^T elementwise, row-reduced. Per-core partial stats
are AllReduce'd across the 8 cores; the AllReduce latency is hidden by
precomputing the raw routed x1 for the first STAGE categories.

Per category: PRelu(a*x1+b) -> x2 (bf16) -> head matmul with x2 as the
stationary operand (out = x2_block^T @ Wc[c], [128 pts, 6]) -> bias add ->
log-softmax over the 6 logits. Host unsorts rows and scatters the <=6
valid segment columns into the [N, 50] output (pure indexing, no math).
"""

import os
import sys
import functools
from contextlib import ExitStack

import numpy as np
import ml_dtypes

BF = ml_dtypes.bfloat16

for _p in ("/opt/trn_rl_repo", "/root/.axon_site/_ro/trn_rl_repo"):
    if os.path.isdir(_p) and _p not in sys.path:
        sys.path.insert(0, _p)

import concourse.bass as bass
import concourse.tile as tile
from concourse import bacc
from concourse import mybir
from concourse.bass_utils import run_bass_kernel_spmd
from concourse.tile_rust import add_dep_helper

NCORES = 8
NPTS = 4096          # points per core
KF = 256             # input features
NCAT = 16
SEG = 6
CAP = 320            # padded points per category (seed-0 max count is 302;
                     # overflow falls back to numpy, prob ~2e-3 per random seed)
NBLK = 3             # head blocks per category: 128 + 128 + 64 points
BLKS = [(0, 128), (128, 256), (256, CAP)]
NRT = NCAT * CAP     # routed+padded points per core
MCH = 32             # 4096 channels / 128
NCH = NCAT * KF
N_GLOBAL = NCORES * NPTS
BN_EPS = 1e-5
LEAK = 0.2

f32 = mybir.dt.float32
bf16 = mybir.dt.bfloat16
fp8 = mybir.dt.float8e4
F8 = ml_dtypes.float8_e4m3fn
KPAD = 272           # fp8 DoubleRow needs Ko step % 16 == 0 (257 -> 272)
W1SC = 16.0          # fp8 scale for W1 entries (~N(0, 1/256))
CSC = 1.0 / 32.0     # fp8 scale for Gram entries (diag ~4096)
AF = mybir.ActivationFunctionType
ALU = mybir.AluOpType
DR = mybir.MatmulPerfMode.DoubleRow


class _Bacc(bacc.Bacc):
    """Pin the single activation table (Prelu + Exp + Ln) so the kernel
    never swaps tables."""

    def insert_act_table_loads(self):
        import bass_rust as _br
        from concourse.hw_specs import get_activation_tables
        has_activation = any(
            isinstance(i, mybir.InstActivation)
            for b in self.main_func.blocks
            for i in b.instructions
        )
        if not has_activation:
            return
        keep = ("natural_log_exp_and_others",)
        tables = [
            (name, funcs if name in keep else set())
            for name, funcs in get_activation_tables(self.m.arch).items()
        ]
        _br.insert_act_table_loads(self, tables)


def build_program():
    nc = _Bacc()

    HMC = MCH // 2
    # fnat8: fp8 features, point-chunks interleaved in DoubleRow pairs
    fnatA_d = nc.dram_tensor("fnatA", [128, 8, 2, KPAD], fp8, kind="ExternalInput")
    fnatB_d = nc.dram_tensor("fnatB", [128, 8, 2, KPAD], fp8, kind="ExternalInput")
    w1f8_d = nc.dram_tensor("w1f8", [128, 2, NCH], fp8, kind="ExternalInput")
    w1_d = nc.dram_tensor("w1", [128, 2, NCH], bf16, kind="ExternalInput")
    w1TA_d = nc.dram_tensor("w1TA", [128, HMC, KF], bf16, kind="ExternalInput")
    w1TB_d = nc.dram_tensor("w1TB", [128, HMC, KF], bf16, kind="ExternalInput")
    featTA_d = nc.dram_tensor("featTA", [128, 2, NRT // 2], bf16, kind="ExternalInput")
    featTB_d = nc.dram_tensor("featTB", [128, 2, NRT // 2], bf16, kind="ExternalInput")
    wc_d = nc.dram_tensor("wc", [128, 2, NCAT, SEG], bf16, kind="ExternalInput")
    gam_d = nc.dram_tensor("gamma_t", [128, MCH], f32, kind="ExternalInput")
    bet_d = nc.dram_tensor("beta_t", [128, MCH], f32, kind="ExternalInput")
    bias18_d = nc.dram_tensor("bias18", [1, NBLK * SEG], f32, kind="ExternalInput")
    # SBUF-native layout: routed point i*128+p lives at out[p, i, :]
    out_d = nc.dram_tensor("out", [128, NCAT * NBLK, SEG], f32, kind="ExternalOutput")
    stats_in_d = nc.dram_tensor("stats_in", [128, 2 * MCH], f32)
    stats_out_d = nc.dram_tensor("stats_out", [128, 2 * MCH], f32, addr_space="Shared")

    with ExitStack() as ctx:
        tc = ctx.enter_context(tile.TileContext(nc))
        big = ctx.enter_context(tc.tile_pool(name="big", bufs=1))
        consts = ctx.enter_context(tc.tile_pool(name="consts", bufs=1))
        stat = ctx.enter_context(tc.tile_pool(name="stat", bufs=1))
        work = ctx.enter_context(tc.tile_pool(name="work", bufs=3))
        x2p = ctx.enter_context(tc.tile_pool(name="x2p", bufs=3))
        outp = ctx.enter_context(tc.tile_pool(name="outp", bufs=3))
        psA = ctx.enter_context(tc.tile_pool(name="psA", bufs=2, space="PSUM"))
        psH = ctx.enter_context(tc.tile_pool(name="psH", bufs=2, space="PSUM"))
        psG = ctx.enter_context(tc.tile_pool(name="psG", bufs=1, space="PSUM"))

        # ---------------- loads, chained so earlier-needed tensors get the
        # full DMA bandwidth first (unchained DMAs share queues round-robin)
        fnatA = big.tile([128, 8, 2, KPAD], fp8)
        nc.sync.dma_start(out=fnatA, in_=fnatA_d[:])
        fnatB = big.tile([128, 8, 2, KPAD], fp8)
        nc.sync.dma_start(out=fnatB, in_=fnatB_d[:])
        w1f8 = big.tile([128, 2, NCH], fp8)
        nc.sync.dma_start(out=w1f8, in_=w1f8_d[:])
        wc = consts.tile([128, 2, NCAT, SEG], bf16)
        nc.sync.dma_start(out=wc, in_=wc_d[:])
        gam = consts.tile([128, MCH], f32)
        nc.sync.dma_start(out=gam, in_=gam_d[:])
        bet = consts.tile([128, MCH], f32)
        nc.sync.dma_start(out=bet, in_=bet_d[:])
        bias18 = consts.tile([128, NBLK * SEG], f32)
        nc.sync.dma_start(out=bias18, in_=bias18_d[:].to_broadcast((128, NBLK * SEG)))

        # ---------------- phase 0: Gram matrix (with ones column -> col sums)
        # fp8 DoubleRow: 256 points contracted per matmul, 2x column rate.
        # pc[kc][l_low, k] = C[kc*128+l_low, k] for k<256; col 256 = s[l]
        pc = psG.tile([128, 2, 512], f32)
        for i2 in range(16):
            fn = fnatA if i2 < 8 else fnatB
            ii = i2 % 8
            for kc in range(2):
                nc.tensor.matmul(
                    pc[:, kc, 0:KF + 1],
                    lhsT=fn[:, ii, :, kc * 128:(kc + 1) * 128],
                    rhs=fn[:, ii, :, 0:KF + 1],
                    start=(i2 == 0),
                    stop=(i2 == 15),
                    perf_mode=DR,
                )
        # C8 = C * CSC in fp8, DoubleRow layout [l_low, l_tile, k]
        C8 = stat.tile([128, 2, KPAD], fp8)
        nc.vector.tensor_scalar_mul(out=C8[:, :, 0:KF + 1], in0=pc[:, :, 0:KF + 1], scalar1=CSC)

        # mid/late loads: issued after the Gram so their descriptors queue
        # behind fnat/w1f8 (FIFO per queue = bandwidth priority)
        w1TA = big.tile([128, HMC, KF], bf16)
        nc.sync.dma_start(out=w1TA, in_=w1TA_d[:])
        w1TB = big.tile([128, HMC, KF], bf16)
        nc.sync.dma_start(out=w1TB, in_=w1TB_d[:])
        w1 = big.tile([128, 2, NCH], bf16)
        nc.sync.dma_start(out=w1, in_=w1_d[:])
        featTA = big.tile([128, 2, NRT // 2], bf16)
        nc.sync.dma_start(out=featTA, in_=featTA_d[:])
        featTB = big.tile([128, 2, NRT // 2], bf16)
        nc.sync.dma_start(out=featTB, in_=featTB_d[:])

        # ---------------- phase 1: D_T = W1^T C (+ sumx1 col), stats
        # pd[j_low, k] = W1SC*CSC * sum_l W1[l, m*128+j_low] C[l, k]
        stats_sb = stat.tile([128, 2 * MCH], f32)
        for m in range(MCH):
            pd = psA.tile([128, 2, 512], f32, tag="x1")
            nc.tensor.matmul(
                pd[:, 0, 0:KF + 1],
                lhsT=w1f8[:, :, m * 128:(m + 1) * 128],
                rhs=C8[:, :, 0:KF + 1],
                start=True,
                stop=True,
                perf_mode=DR,
            )
            nc.vector.tensor_copy(
                out=stats_sb[:, m:m + 1], in_=pd[:, 0, KF:KF + 1]
            )
            w1Tt = w1TA if m < MCH // 2 else w1TB
            pt = work.tile([128, KF], f32, tag="pt")
            nc.vector.tensor_mul(
                out=pt, in0=pd[:, 0, 0:KF], in1=w1Tt[:, m % (MCH // 2), :]
            )
            nc.vector.tensor_reduce(
                out=stats_sb[:, MCH + m:MCH + m + 1], in_=pt,
                axis=mybir.AxisListType.X, op=ALU.add,
            )

        wr = nc.sync.dma_start(out=stats_in_d[:], in_=stats_sb)
        cc = nc.gpsimd.collective_compute(
            "AllReduce",
            ALU.add,
            replica_groups=[list(range(NCORES))],
            ins=[stats_in_d[:]],
            outs=[stats_out_d[:]],
        )
        add_dep_helper(cc.ins, wr.ins, reason="stats written before allreduce")

        # ---------------- overlap AllReduce: raw routed x1 for ALL cats
        def x1_matmuls(px, c):
            ft = featTA if c < NCAT // 2 else featTB
            c0 = (c % (NCAT // 2)) * CAP
            for jh in range(2):
                for kc in range(2):
                    nc.tensor.matmul(
                        px[:, jh, 0:CAP],
                        lhsT=w1[:, kc, c * KF + jh * 128: c * KF + (jh + 1) * 128],
                        rhs=ft[:, kc, c0:c0 + CAP],
                        start=(kc == 0),
                        stop=(kc == 1),
                    )

        x1sb = big.tile([128, 2, NRT], bf16)
        for c in range(NCAT):
            px = psA.tile([128, 2, 512], f32, tag="x1")
            x1_matmuls(px, c)
            nc.vector.tensor_copy(
                out=x1sb[:, :, c * CAP:(c + 1) * CAP], in_=px[:, :, 0:CAP]
            )

        stats_g = stat.tile([128, 2 * MCH], f32)
        rd = nc.sync.dma_start(out=stats_g, in_=stats_out_d[:])
        add_dep_helper(rd.ins, cc.ins, reason="allreduce before readback")

        # ---------------- BN affine params a, b per channel
        SSC = 1.0 / (W1SC * CSC * N_GLOBAL)   # undo fp8 scaling of D_T
        mu = stat.tile([128, MCH], f32)
        nc.vector.tensor_scalar(
            out=mu, in0=stats_g[:, 0:MCH], scalar1=SSC, scalar2=None,
            op0=ALU.mult,
        )
        var = stat.tile([128, MCH], f32)
        nc.vector.tensor_scalar(
            out=var, in0=stats_g[:, MCH:2 * MCH], scalar1=SSC,
            scalar2=None, op0=ALU.mult,
        )
        mu2 = stat.tile([128, MCH], f32)
        nc.vector.tensor_mul(out=mu2, in0=mu, in1=mu)
        nc.vector.tensor_sub(out=var, in0=var, in1=mu2)
        eps_t = stat.tile([128, 1], f32)
        nc.vector.memset(eps_t, BN_EPS)
        # rstd = exp(-0.5 * ln(var + eps)) -- stays in the one act table
        lnv = stat.tile([128, MCH], f32)
        nc.scalar.activation(out=lnv, in_=var, func=AF.Ln, bias=eps_t, scale=1.0)
        rstd = stat.tile([128, MCH], f32)
        nc.scalar.activation(out=rstd, in_=lnv, func=AF.Exp, scale=-0.5)
        a_t = stat.tile([128, MCH], f32)
        nc.vector.tensor_mul(out=a_t, in0=gam, in1=rstd)
        b_t = stat.tile([128, MCH], f32)
        nc.vector.tensor_mul(out=b_t, in0=mu, in1=a_t)
        nc.vector.tensor_sub(out=b_t, in0=bet, in1=b_t)

        # ---------------- main loop: per-category normalize + head + lsm
        obuf = big.tile([128, NCAT * NBLK, SEG], f32)
        for c in range(NCAT):
            x1src = x1sb[:, :, c * CAP:(c + 1) * CAP]
            x2 = x2p.tile([128, 2, CAP], bf16, tag="x2")
            for jh in range(2):
                m = 2 * c + jh
                if m % 8 < 5:
                    # scalar-engine path
                    nc.scalar.activation(
                        out=x2[:, jh, :], in_=x1src[:, jh, :], func=AF.Prelu,
                        bias=b_t[:, m:m + 1], scale=a_t[:, m:m + 1], alpha=LEAK,
                    )
                else:
                    # vector-engine path: y = a*x1+b; x2 = max(y, 0.2y)
                    y = work.tile([128, CAP], bf16, tag="y")
                    nc.vector.tensor_scalar(
                        out=y, in0=x1src[:, jh, :], scalar1=a_t[:, m:m + 1],
                        scalar2=b_t[:, m:m + 1], op0=ALU.mult, op1=ALU.add,
                    )
                    y2 = work.tile([128, CAP], bf16, tag="y2")
                    nc.vector.tensor_scalar_mul(out=y2, in0=y, scalar1=LEAK)
                    nc.vector.tensor_tensor(
                        out=x2[:, jh, :], in0=y, in1=y2, op=ALU.max,
                    )
            # head: psum initialized with the shared bias, matmuls accumulate
            ph = psH.tile([128, NBLK, SEG], f32, tag="ph")
            nc.vector.tensor_copy(out=ph, in_=bias18)
            for blk, (b0, b1) in enumerate(BLKS):
                for jh in range(2):
                    nc.tensor.matmul(
                        ph[0:b1 - b0, blk, :],
                        lhsT=x2[:, jh, b0:b1],
                        rhs=wc[:, jh, c, :],
                        start=False,
                        stop=(jh == 1),
                        skip_group_check=True,
                    )
            e = outp.tile([128, NBLK, SEG], f32, tag="e")
            nc.scalar.activation(out=e, in_=ph, func=AF.Exp)
            se = outp.tile([128, NBLK], f32, tag="se")
            nc.vector.tensor_reduce(
                out=se, in_=e, axis=mybir.AxisListType.X, op=ALU.add,
            )
            lse = outp.tile([128, NBLK, 1], f32, tag="lse")
            nc.scalar.activation(out=lse, in_=se, func=AF.Ln)
            nc.vector.tensor_tensor(
                out=obuf[:, c * NBLK:(c + 1) * NBLK, :], in0=ph,
                in1=lse.to_broadcast((128, NBLK, SEG)),
                op=ALU.subtract,
            )
        nc.sync.dma_start(out=out_d[:], in_=obuf)

    if not nc.is_finalized():
        nc.finalize()
    return nc


@functools.lru_cache(maxsize=1)
def _get_program():
    return build_program()


def _ref_numpy(features, W1, gamma, beta, Wc, bias, cats, shifts, seg_lens):
    """Pure-numpy fallback, only used if a category exceeds CAP (never for
    realistic inputs; probability ~1e-11 for uniform random cats)."""
    x = features @ W1
    mu = x.mean(0)
    var = x.var(0)
    x = (x - mu) / np.sqrt(var + BN_EPS) * gamma + beta
    x = np.where(x >= 0, x, LEAK * x)
    N = features.shape[0]
    x = x.reshape(N, NCAT, KF)
    xg = x[np.arange(N), cats]                      # [N, KF]
    logits = np.einsum('nf,nfs->ns', xg, Wc[cats]) + bias
    m = logits.max(1, keepdims=True)
    lsm = logits - m - np.log(np.exp(logits - m).sum(1, keepdims=True))
    return lsm


def _scatter50(lsm_all, cats, shifts, seg_lens):
    N = lsm_all.shape[0]
    sh = shifts[cats]
    ln = seg_lens[cats]
    k = np.arange(50)
    j = k[None, :] - sh[:, None]
    valid = (j >= 0) & (j < ln[:, None])
    jc = np.clip(j, 0, SEG - 1)
    return np.where(valid, np.take_along_axis(lsm_all, jc, axis=1), 0.0)


def _host_prep(features, W1, cats):
    features = np.ascontiguousarray(np.asarray(features, dtype=np.float32))
    W1 = np.ascontiguousarray(np.asarray(W1, dtype=np.float32))
    cats = np.asarray(cats)

    in_maps = []
    orders = []
    counts_all = []
    for ci in range(NCORES):
        fc = features[ci * NPTS:(ci + 1) * NPTS]
        cc = cats[ci * NPTS:(ci + 1) * NPTS]
        order = np.argsort(cc, kind="stable")
        counts = np.bincount(cc, minlength=NCAT)
        if counts.max() > CAP:
            return None, None, None
        orders.append(order)
        counts_all.append(counts)

        ft_rt = np.zeros((NRT, KF), np.float32)
        start = 0
        for c in range(NCAT):
            n = int(counts[c])
            ft_rt[c * CAP:c * CAP + n] = fc[order[start:start + n]]
            start += n
        featT = np.ascontiguousarray(
            ft_rt.T.reshape(2, 128, NRT).transpose(1, 0, 2)
        ).astype(BF)
        # DoubleRow fp8 layout: [p, pair, tile, k], point = (pair*2+tile)*128+p
        fnat8 = np.zeros((128, 16, 2, KPAD), F8)
        fnat8[:, :, :, 0:KF + 1] = (
            np.concatenate([fc, np.ones((NPTS, 1), np.float32)], axis=1)
            .reshape(16, 2, 128, KF + 1).transpose(2, 0, 1, 3)
        ).astype(F8)
        m = {
            "featTA": np.ascontiguousarray(featT[:, :, 0:NRT // 2]),
            "featTB": np.ascontiguousarray(featT[:, :, NRT // 2:]),
            "fnatA": np.ascontiguousarray(fnat8[:, 0:8]),
            "fnatB": np.ascontiguousarray(fnat8[:, 8:]),
        }
        in_maps.append(m)
    return in_maps, orders, counts_all


def _host_prep_common(W1, gamma, beta, Wc, bias):
    W1 = np.ascontiguousarray(np.asarray(W1, dtype=np.float32))
    gamma = np.asarray(gamma, dtype=np.float32)
    beta = np.asarray(beta, dtype=np.float32)
    Wc = np.asarray(Wc, dtype=np.float32)
    bias = np.asarray(bias, dtype=np.float32)
    w1T = np.ascontiguousarray(
        W1.T.reshape(MCH, 128, KF).transpose(1, 0, 2)
    ).astype(BF)
    w1p = np.ascontiguousarray(W1.reshape(2, 128, NCH).transpose(1, 0, 2))
    return {
        "w1": w1p.astype(BF),
        "w1f8": (w1p * W1SC).astype(F8),
        "w1TA": np.ascontiguousarray(w1T[:, 0:MCH // 2]),
        "w1TB": np.ascontiguousarray(w1T[:, MCH // 2:]),
        "wc": np.ascontiguousarray(
            Wc.transpose(1, 0, 2).reshape(2, 128, NCAT, SEG)
            .transpose(1, 0, 2, 3)
        ).astype(BF),
        "gamma_t": np.ascontiguousarray(gamma.reshape(MCH, 128).T),
        "beta_t": np.ascontiguousarray(beta.reshape(MCH, 128).T),
        "bias18": np.tile(bias, NBLK).astype(np.float32).reshape(1, NBLK * SEG),
    }


def _run(inputs, trace=False):
    features = np.asarray(inputs["features"], dtype=np.float32)
    W1 = np.asarray(inputs["W1"], dtype=np.float32)
    cats = np.asarray(inputs["cats"])
    shifts = np.asarray(inputs["shifts"]).astype(np.int64)
    seg_lens = np.asarray(inputs["seg_lens"]).astype(np.int64)

    in_maps, orders, counts_all = _host_prep(features, W1, cats)
    if in_maps is None:
        lsm = _ref_numpy(
            features, W1, np.asarray(inputs["gamma"], np.float32),
            np.asarray(inputs["beta"], np.float32),
            np.asarray(inputs["Wc"], np.float32),
            np.asarray(inputs["bias"], np.float32), cats, shifts, seg_lens,
        )
        out = _scatter50(lsm, cats, shifts, seg_lens).astype(np.float32)
        return out, None
    common = _host_prep_common(
        W1, inputs["gamma"], inputs["beta"], inputs["Wc"], inputs["bias"]
    )
    for m in in_maps:
        m.update(common)

    nc = _get_program()
    res = run_bass_kernel_spmd(
        nc, in_maps, core_ids=list(range(NCORES)), trace=trace
    )

    lsm_all = np.empty((NCORES * NPTS, SEG), np.float32)
    for ci in range(NCORES):
        arr = np.asarray(res.results[ci]["out"], np.float32)
        routed = np.empty((NRT, SEG), np.float32)
        for c in range(NCAT):
            for blk, (b0, b1) in enumerate(BLKS):
                routed[c * CAP + b0:c * CAP + b1] = arr[0:b1 - b0, c * NBLK + blk]
        order = orders[ci]
        counts = counts_all[ci]
        full = np.empty((NPTS, SEG), np.float32)
        start = 0
        for c in range(NCAT):
            n = int(counts[c])
            full[order[start:start + n]] = routed[c * CAP:c * CAP + n]
            start += n
        lsm_all[ci * NPTS:(ci + 1) * NPTS] = full

    out = _scatter50(lsm_all, cats, shifts, seg_lens).astype(np.float32)
    return np.ascontiguousarray(out), res


def kernel(**inputs):
    out, _ = _run(inputs, trace=False)
    return out


# used by test.py for profiling runs
def kernel_traced(**inputs):
    out, res = _run(inputs, trace=True)
    return out, res


# revision 31
# speedup vs baseline: 1.2474x; 1.2474x over previous
"""Trainium2 Bass kernel for nn_MultiHeadClassifier (moe_routing).

Strategy: data-parallel over N=32768 points across 8 NeuronCores (4096
points/core), then EXPERT ROUTING within each core: the host stably sorts
each core's points by category and pads every category to CAP=384 slots.
The device then computes, per point, only its own category's 256-channel
block of x1 = feat @ W1 (16x less PE work than the dense 4096-channel
version) plus that category's 6-way head.

BatchNorm batch statistics over the FULL 4096 channels are computed
analytically from the 256x256 feature Gram matrix:
  sumx1[j]  = s^T W1[:, j]          (s = per-feature column sums)
  sumsq[j]  = W1[:, j]^T C W1[:, j] (C = F^T F)
via D_T = W1^T C (with an extra ones-column of the Gram giving s^T W1 for
free), P_T = D_T * W1^T elementwise, row-reduced. Per-core partial stats
are AllReduce'd across the 8 cores; the AllReduce latency is hidden by
precomputing the raw routed x1 for the first STAGE categories.

Per category: PRelu(a*x1+b) -> x2 (bf16) -> head matmul with x2 as the
stationary operand (out = x2_block^T @ Wc[c], [128 pts, 6]) -> bias add ->
log-softmax over the 6 logits. Host unsorts rows and scatters the <=6
valid segment columns into the [N, 50] output (pure indexing, no math).
"""

import os
import sys
import functools
from contextlib import ExitStack

import numpy as np
import ml_dtypes

BF = ml_dtypes.bfloat16

for _p in ("/opt/trn_rl_repo", "/root/.axon_site/_ro/trn_rl_repo"):
    if os.path.isdir(_p) and _p not in sys.path:
        sys.path.insert(0, _p)

import concourse.bass as bass
import concourse.tile as tile
from concourse import bacc
from concourse import mybir
from concourse.bass_utils import run_bass_kernel_spmd
from concourse.tile_rust import add_dep_helper

NCORES = 8
NPTS = 4096          # points per core
KF = 256             # input features
NCAT = 16
SEG = 6
CAP = 320            # padded points per category (seed-0 max count is 302;
                     # overflow falls back to numpy, prob ~2e-3 per random seed)
NBLK = 3             # head blocks per category: 128 + 128 + 64 points
BLKS = [(0, 128), (128, 256), (256, CAP)]
NRT = NCAT * CAP     # routed+padded points per core
MCH = 32             # 4096 channels / 128
NCH = NCAT * KF
N_GLOBAL = NCORES * NPTS
BN_EPS = 1e-5
LEAK = 0.2

f32 = mybir.dt.float32
bf16 = mybir.dt.bfloat16
fp8 = mybir.dt.float8e4
F8 = ml_dtypes.float8_e4m3fn
KPAD = 272           # fp8 DoubleRow needs Ko step % 16 == 0 (257 -> 272)
W1SC = 16.0          # fp8 scale for W1 entries (~N(0, 1/256))
CSC = 1.0 / 32.0     # fp8 scale for Gram entries (diag ~4096)
AF = mybir.ActivationFunctionType
ALU = mybir.AluOpType
DR = mybir.MatmulPerfMode.DoubleRow


class _Bacc(bacc.Bacc):
    """Pin the single activation table (Prelu + Exp + Ln) so the kernel
    never swaps tables."""

    def insert_act_table_loads(self):
        import bass_rust as _br
        from concourse.hw_specs import get_activation_tables
        has_activation = any(
            isinstance(i, mybir.InstActivation)
            for b in self.main_func.blocks
            for i in b.instructions
        )
        if not has_activation:
            return
        keep = ("natural_log_exp_and_others",)
        tables = [
            (name, funcs if name in keep else set())
            for name, funcs in get_activation_tables(self.m.arch).items()
        ]
        _br.insert_act_table_loads(self, tables)


def build_program():
    nc = _Bacc()

    HMC = MCH // 2
    # fnat8: fp8 features, point-chunks interleaved in DoubleRow pairs
    fnatA_d = nc.dram_tensor("fnatA", [128, 8, 2, KPAD], fp8, kind="ExternalInput")
    fnatB_d = nc.dram_tensor("fnatB", [128, 8, 2, KPAD], fp8, kind="ExternalInput")
    w1f8_d = nc.dram_tensor("w1f8", [128, 2, NCH], fp8, kind="ExternalInput")
    w1_d = nc.dram_tensor("w1", [128, 2, NCH], bf16, kind="ExternalInput")
    w1TA_d = nc.dram_tensor("w1TA", [128, HMC, KF], bf16, kind="ExternalInput")
    w1TB_d = nc.dram_tensor("w1TB", [128, HMC, KF], bf16, kind="ExternalInput")
    featTA_d = nc.dram_tensor("featTA", [128, 2, NRT // 2], bf16, kind="ExternalInput")
    featTB_d = nc.dram_tensor("featTB", [128, 2, NRT // 2], bf16, kind="ExternalInput")
    wc_d = nc.dram_tensor("wc", [128, 2, NCAT, SEG], bf16, kind="ExternalInput")
    gam_d = nc.dram_tensor("gamma_t", [128, MCH], f32, kind="ExternalInput")
    bet_d = nc.dram_tensor("beta_t", [128, MCH], f32, kind="ExternalInput")
    bias18_d = nc.dram_tensor("bias18", [1, NBLK * SEG], f32, kind="ExternalInput")
    # SBUF-native layout: routed point i*128+p lives at out[p, i, :]
    out_d = nc.dram_tensor("out", [128, NCAT * NBLK, SEG], f32, kind="ExternalOutput")
    stats_in_d = nc.dram_tensor("stats_in", [128, 2 * MCH], f32)
    stats_out_d = nc.dram_tensor("stats_out", [128, 2 * MCH], f32, addr_space="Shared")

    with ExitStack() as ctx:
        tc = ctx.enter_context(tile.TileContext(nc))
        big = ctx.enter_context(tc.tile_pool(name="big", bufs=1))
        consts = ctx.enter_context(tc.tile_pool(name="consts", bufs=1))
        stat = ctx.enter_context(tc.tile_pool(name="stat", bufs=1))
        work = ctx.enter_context(tc.tile_pool(name="work", bufs=3))
        x2p = ctx.enter_context(tc.tile_pool(name="x2p", bufs=3))
        outp = ctx.enter_context(tc.tile_pool(name="outp", bufs=3))
        psA = ctx.enter_context(tc.tile_pool(name="psA", bufs=2, space="PSUM"))
        psH = ctx.enter_context(tc.tile_pool(name="psH", bufs=2, space="PSUM"))
        psG = ctx.enter_context(tc.tile_pool(name="psG", bufs=1, space="PSUM"))

        # ---------------- loads, chained so earlier-needed tensors get the
        # full DMA bandwidth first (unchained DMAs share queues round-robin)
        fnatA = big.tile([128, 8, 2, KPAD], fp8)
        nc.sync.dma_start(out=fnatA, in_=fnatA_d[:])
        fnatB = big.tile([128, 8, 2, KPAD], fp8)
        nc.sync.dma_start(out=fnatB, in_=fnatB_d[:])
        w1f8 = big.tile([128, 2, NCH], fp8)
        nc.sync.dma_start(out=w1f8, in_=w1f8_d[:])
        wc = consts.tile([128, 2, NCAT, SEG], bf16)
        nc.sync.dma_start(out=wc, in_=wc_d[:])
        gam = consts.tile([128, MCH], f32)
        nc.sync.dma_start(out=gam, in_=gam_d[:])
        bet = consts.tile([128, MCH], f32)
        nc.sync.dma_start(out=bet, in_=bet_d[:])
        bias18 = consts.tile([128, NBLK * SEG], f32)
        nc.sync.dma_start(out=bias18, in_=bias18_d[:].to_broadcast((128, NBLK * SEG)))

        # ---------------- phase 0: Gram matrix (with ones column -> col sums)
        # fp8 DoubleRow: 256 points contracted per matmul, 2x column rate.
        # pc[kc][l_low, k] = C[kc*128+l_low, k] for k<256; col 256 = s[l]
        pc = psG.tile([128, 2, 512], f32)
        for i2 in range(16):
            fn = fnatA if i2 < 8 else fnatB
            ii = i2 % 8
            for kc in range(2):
                nc.tensor.matmul(
                    pc[:, kc, 0:KF + 1],
                    lhsT=fn[:, ii, :, kc * 128:(kc + 1) * 128],
                    rhs=fn[:, ii, :, 0:KF + 1],
                    start=(i2 == 0),
                    stop=(i2 == 15),
                    perf_mode=DR,
                )
        # C8 = C * CSC in fp8, DoubleRow layout [l_low, l_tile, k]
        C8 = stat.tile([128, 2, KPAD], fp8)
        nc.vector.tensor_scalar_mul(out=C8[:, :, 0:KF + 1], in0=pc[:, :, 0:KF + 1], scalar1=CSC)

        # mid/late loads: issued after the Gram so their descriptors queue
        # behind fnat/w1f8 (FIFO per queue = bandwidth priority)
        w1TA = big.tile([128, HMC, KF], bf16)
        nc.sync.dma_start(out=w1TA, in_=w1TA_d[:])
        w1TB = big.tile([128, HMC, KF], bf16)
        nc.sync.dma_start(out=w1TB, in_=w1TB_d[:])
        w1 = big.tile([128, 2, NCH], bf16)
        nc.sync.dma_start(out=w1, in_=w1_d[:])
        featTA = big.tile([128, 2, NRT // 2], bf16)
        nc.sync.dma_start(out=featTA, in_=featTA_d[:])
        featTB = big.tile([128, 2, NRT // 2], bf16)
        nc.sync.dma_start(out=featTB, in_=featTB_d[:])

        # ---------------- phase 1: D_T = W1^T C (+ sumx1 col), stats
        # pd[j_low, k] = W1SC*CSC * sum_l W1[l, m*128+j_low] C[l, k]
        stats_sb = stat.tile([128, 2 * MCH], f32)
        for m in range(MCH):
            pd = psA.tile([128, 2, 512], f32, tag="x1")
            nc.tensor.matmul(
                pd[:, 0, 0:KF + 1],
                lhsT=w1f8[:, :, m * 128:(m + 1) * 128],
                rhs=C8[:, :, 0:KF + 1],
                start=True,
                stop=True,
                perf_mode=DR,
            )
            nc.vector.tensor_copy(
                out=stats_sb[:, m:m + 1], in_=pd[:, 0, KF:KF + 1]
            )
            w1Tt = w1TA if m < MCH // 2 else w1TB
            pt = work.tile([128, KF], f32, tag="pt")
            nc.vector.tensor_mul(
                out=pt, in0=pd[:, 0, 0:KF], in1=w1Tt[:, m % (MCH // 2), :]
            )
            nc.vector.tensor_reduce(
                out=stats_sb[:, MCH + m:MCH + m + 1], in_=pt,
                axis=mybir.AxisListType.X, op=ALU.add,
            )

        wr = nc.sync.dma_start(out=stats_in_d[:], in_=stats_sb)
        cc = nc.gpsimd.collective_compute(
            "AllReduce",
            ALU.add,
            replica_groups=[list(range(NCORES))],
            ins=[stats_in_d[:]],
            outs=[stats_out_d[:]],
        )
        add_dep_helper(cc.ins, wr.ins, reason="stats written before allreduce")

        # ---------------- overlap AllReduce: raw routed x1 for ALL cats
        def x1_matmuls(px, c):
            ft = featTA if c < NCAT // 2 else featTB
            c0 = (c % (NCAT // 2)) * CAP
            for jh in range(2):
                for kc in range(2):
                    nc.tensor.matmul(
                        px[:, jh, 0:CAP],
                        lhsT=w1[:, kc, c * KF + jh * 128: c * KF + (jh + 1) * 128],
                        rhs=ft[:, kc, c0:c0 + CAP],
                        start=(kc == 0),
                        stop=(kc == 1),
                    )

        x1sb = big.tile([128, 2, NRT], bf16)
        for c in range(NCAT):
            px = psA.tile([128, 2, 512], f32, tag="x1")
            x1_matmuls(px, c)
            nc.vector.tensor_copy(
                out=x1sb[:, :, c * CAP:(c + 1) * CAP], in_=px[:, :, 0:CAP]
            )

        stats_g = stat.tile([128, 2 * MCH], f32)
        rd = nc.sync.dma_start(out=stats_g, in_=stats_out_d[:])
        add_dep_helper(rd.ins, cc.ins, reason="allreduce before readback")

        # ---------------- BN affine params a, b per channel
        SSC = 1.0 / (W1SC * CSC * N_GLOBAL)   # undo fp8 scaling of D_T
        mu = stat.tile([128, MCH], f32)
        nc.vector.tensor_scalar(
            out=mu, in0=stats_g[:, 0:MCH], scalar1=SSC, scalar2=None,
            op0=ALU.mult,
        )
        var = stat.tile([128, MCH], f32)
        nc.vector.tensor_scalar(
            out=var, in0=stats_g[:, MCH:2 * MCH], scalar1=SSC,
            scalar2=None, op0=ALU.mult,
        )
        mu2 = stat.tile([128, MCH], f32)
        nc.vector.tensor_mul(out=mu2, in0=mu, in1=mu)
        nc.vector.tensor_sub(out=var, in0=var, in1=mu2)
        eps_t = stat.tile([128, 1], f32)
        nc.vector.memset(eps_t, BN_EPS)
        # rstd = exp(-0.5 * ln(var + eps)) -- stays in the one act table
        lnv = stat.tile([128, MCH], f32)
        nc.scalar.activation(out=lnv, in_=var, func=AF.Ln, bias=eps_t, scale=1.0)
        rstd = stat.tile([128, MCH], f32)
        nc.scalar.activation(out=rstd, in_=lnv, func=AF.Exp, scale=-0.5)
        a_t = stat.tile([128, MCH], f32)
        nc.vector.tensor_mul(out=a_t, in0=gam, in1=rstd)
        b_t = stat.tile([128, MCH], f32)
        nc.vector.tensor_mul(out=b_t, in0=mu, in1=a_t)
        nc.vector.tensor_sub(out=b_t, in0=bet, in1=b_t)

        # ---------------- main loop: per-category normalize + head + lsm
        obuf = big.tile([128, NCAT * NBLK, SEG], f32)
        for c in range(NCAT):
            x1src = x1sb[:, :, c * CAP:(c + 1) * CAP]
            x2 = x2p.tile([128, 2, CAP], bf16, tag="x2")
            for jh in range(2):
                m = 2 * c + jh
                if m % 8 < 5:
                    # scalar-engine path
                    nc.scalar.activation(
                        out=x2[:, jh, :], in_=x1src[:, jh, :], func=AF.Prelu,
                        bias=b_t[:, m:m + 1], scale=a_t[:, m:m + 1], alpha=LEAK,
                    )
                else:
                    # vector-engine path: y = a*x1+b; x2 = max(y, 0.2y)
                    y = work.tile([128, CAP], bf16, tag="y")
                    nc.vector.tensor_scalar(
                        out=y, in0=x1src[:, jh, :], scalar1=a_t[:, m:m + 1],
                        scalar2=b_t[:, m:m + 1], op0=ALU.mult, op1=ALU.add,
                    )
                    y2 = work.tile([128, CAP], bf16, tag="y2")
                    nc.vector.tensor_scalar_mul(out=y2, in0=y, scalar1=LEAK)
                    nc.vector.tensor_tensor(
                        out=x2[:, jh, :], in0=y, in1=y2, op=ALU.max,
                    )
            # head: psum initialized with the shared bias, matmuls accumulate
            ph = psH.tile([128, NBLK, SEG], f32, tag="ph")
            nc.vector.tensor_copy(out=ph, in_=bias18)
            for blk, (b0, b1) in enumerate(BLKS):
                for jh in range(2):
                    nc.tensor.matmul(
                        ph[0:b1 - b0, blk, :],
                        lhsT=x2[:, jh, b0:b1],
                        rhs=wc[:, jh, c, :],
                        start=False,
                        stop=(jh == 1),
                        skip_group_check=True,
                    )
            e = outp.tile([128, NBLK, SEG], f32, tag="e")
            nc.scalar.activation(out=e, in_=ph, func=AF.Exp)
            se = outp.tile([128, NBLK], f32, tag="se")
            nc.vector.tensor_reduce(
                out=se, in_=e, axis=mybir.AxisListType.X, op=ALU.add,
            )
            lse = outp.tile([128, NBLK, 1], f32, tag="lse")
            nc.scalar.activation(out=lse, in_=se, func=AF.Ln)
            nc.vector.tensor_tensor(
                out=obuf[:, c * NBLK:(c + 1) * NBLK, :], in0=ph,
                in1=lse.to_broadcast((128, NBLK, SEG)),
                op=ALU.subtract,
            )
        nc.sync.dma_start(out=out_d[:], in_=obuf)

    if not nc.is_finalized():
        nc.finalize()
    return nc


@functools.lru_cache(maxsize=1)
def _get_program():
    return build_program()


def _ref_numpy(features, W1, gamma, beta, Wc, bias, cats, shifts, seg_lens):
    """Pure-numpy fallback, only used if a category exceeds CAP (never for
    realistic inputs; probability ~1e-11 for uniform random cats)."""
    x = features @ W1
    mu = x.mean(0)
    var = x.var(0)
    x = (x - mu) / np.sqrt(var + BN_EPS) * gamma + beta
    x = np.where(x >= 0, x, LEAK * x)
    N = features.shape[0]
    x = x.reshape(N, NCAT, KF)
    xg = x[np.arange(N), cats]                      # [N, KF]
    logits = np.einsum('nf,nfs->ns', xg, Wc[cats]) + bias
    m = logits.max(1, keepdims=True)
    lsm = logits - m - np.log(np.exp(logits - m).sum(1, keepdims=True))
    return lsm


def _scatter50(lsm_all, cats, shifts, seg_lens):
    N = lsm_all.shape[0]
    sh = shifts[cats]
    ln = seg_lens[cats]
    k = np.arange(50)
    j = k[None, :] - sh[:, None]
    valid = (j >= 0) & (j < ln[:, None])
    jc = np.clip(j, 0, SEG - 1)
    return np.where(valid, np.take_along_axis(lsm_all, jc, axis=1), 0.0)


def _host_prep(features, W1, cats):
    features = np.ascontiguousarray(np.asarray(features, dtype=np.float32))
    W1 = np.ascontiguousarray(np.asarray(W1, dtype=np.float32))
    cats = np.asarray(cats)

    in_maps = []
    orders = []
    counts_all = []
    for ci in range(NCORES):
        fc = features[ci * NPTS:(ci + 1) * NPTS]
        cc = cats[ci * NPTS:(ci + 1) * NPTS]
        order = np.argsort(cc, kind="stable")
        counts = np.bincount(cc, minlength=NCAT)
        if counts.max() > CAP:
            return None, None, None
        orders.append(order)
        counts_all.append(counts)

        ft_rt = np.zeros((NRT, KF), np.float32)
        start = 0
        for c in range(NCAT):
            n = int(counts[c])
            ft_rt[c * CAP:c * CAP + n] = fc[order[start:start + n]]
            start += n
        featT = np.ascontiguousarray(
            ft_rt.T.reshape(2, 128, NRT).transpose(1, 0, 2)
        ).astype(BF)
        # DoubleRow fp8 layout: [p, pair, tile, k], point = (pair*2+tile)*128+p
        fnat8 = np.zeros((128, 16, 2, KPAD), F8)
        fnat8[:, :, :, 0:KF + 1] = (
            np.concatenate([fc, np.ones((NPTS, 1), np.float32)], axis=1)
            .reshape(16, 2, 128, KF + 1).transpose(2, 0, 1, 3)
        ).astype(F8)
        m = {
            "featTA": np.ascontiguousarray(featT[:, :, 0:NRT // 2]),
            "featTB": np.ascontiguousarray(featT[:, :, NRT // 2:]),
            "fnatA": np.ascontiguousarray(fnat8[:, 0:8]),
            "fnatB": np.ascontiguousarray(fnat8[:, 8:]),
        }
        in_maps.append(m)
    return in_maps, orders, counts_all


def _host_prep_common(W1, gamma, beta, Wc, bias):
    W1 = np.ascontiguousarray(np.asarray(W1, dtype=np.float32))
    gamma = np.asarray(gamma, dtype=np.float32)
    beta = np.asarray(beta, dtype=np.float32)
    Wc = np.asarray(Wc, dtype=np.float32)
    bias = np.asarray(bias, dtype=np.float32)
    w1T = np.ascontiguousarray(
        W1.T.reshape(MCH, 128, KF).transpose(1, 0, 2)
    ).astype(BF)
    w1p = np.ascontiguousarray(W1.reshape(2, 128, NCH).transpose(1, 0, 2))
    return {
        "w1": w1p.astype(BF),
        "w1f8": (w1p * W1SC).astype(F8),
        "w1TA": np.ascontiguousarray(w1T[:, 0:MCH // 2]),
        "w1TB": np.ascontiguousarray(w1T[:, MCH // 2:]),
        "wc": np.ascontiguousarray(
            Wc.transpose(1, 0, 2).reshape(2, 128, NCAT, SEG)
            .transpose(1, 0, 2, 3)
        ).astype(BF),
        "gamma_t": np.ascontiguousarray(gamma.reshape(MCH, 128).T),
        "beta_t": np.ascontiguousarray(beta.reshape(MCH, 128).T),
        "bias18": np.tile(bias, NBLK).astype(np.float32).reshape(1, NBLK * SEG),
    }


def _run(inputs, trace=False):
    features = np.asarray(inputs["features"], dtype=np.float32)
    W1 = np.asarray(inputs["W1"], dtype=np.float32)
    cats = np.asarray(inputs["cats"])
    shifts = np.asarray(inputs["shifts"]).astype(np.int64)
    seg_lens = np.asarray(inputs["seg_lens"]).astype(np.int64)

    in_maps, orders, counts_all = _host_prep(features, W1, cats)
    if in_maps is None:
        lsm = _ref_numpy(
            features, W1, np.asarray(inputs["gamma"], np.float32),
            np.asarray(inputs["beta"], np.float32),
            np.asarray(inputs["Wc"], np.float32),
            np.asarray(inputs["bias"], np.float32), cats, shifts, seg_lens,
        )
        out = _scatter50(lsm, cats, shifts, seg_lens).astype(np.float32)
        return out, None
    common = _host_prep_common(
        W1, inputs["gamma"], inputs["beta"], inputs["Wc"], inputs["bias"]
    )
    for m in in_maps:
        m.update(common)

    nc = _get_program()
    res = run_bass_kernel_spmd(
        nc, in_maps, core_ids=list(range(NCORES)), trace=trace
    )

    lsm_all = np.empty((NCORES * NPTS, SEG), np.float32)
    for ci in range(NCORES):
        arr = np.asarray(res.results[ci]["out"], np.float32)
        routed = np.empty((NRT, SEG), np.float32)
        for c in range(NCAT):
            for blk, (b0, b1) in enumerate(BLKS):
                routed[c * CAP + b0:c * CAP + b1] = arr[0:b1 - b0, c * NBLK + blk]
        order = orders[ci]
        counts = counts_all[ci]
        full = np.empty((NPTS, SEG), np.float32)
        start = 0
        for c in range(NCAT):
            n = int(counts[c])
            full[order[start:start + n]] = routed[c * CAP:c * CAP + n]
            start += n
        lsm_all[ci * NPTS:(ci + 1) * NPTS] = full

    out = _scatter50(lsm_all, cats, shifts, seg_lens).astype(np.float32)
    return np.ascontiguousarray(out), res


def kernel(**inputs):
    out, _ = _run(inputs, trace=False)
    return out


# used by test.py for profiling runs
def kernel_traced(**inputs):
    out, res = _run(inputs, trace=True)
    return out, res
